# revision 11
# baseline (speedup 1.0000x reference)
"""FORCE/RLS training step (nn_DMCell) on 8 Trainium2 NeuronCores.

Data-parallel over batch: x/s/y rows sharded across 8 cores; P/win/wr
replicated (fed pre-transposed + TF32-rounded for the fp32r matmul path).
The rank-k updates k.T@k_fenmu and k.T@err are computed as per-core
partial GEMMs, reduce-scattered across cores, and each core emits its
row-slice of P_new.T / win_new.T; the host reassembles + transposes.

Math (per reference):
  rx   = x @ win.T + s @ wr.T
  err  = rx - y
  kf   = x @ P.T ; rPr = rowsum(kf*x) ; k = kf/(1+rPr)
  win' = win - (k.T @ err).T / B
  P'   = P - (k.T @ kf) / B
  r    = z/(D*(1-exp(-z))), z = D*(A*rx - B_WW)   [stable: z*ez/(D*(ez-1))]
  s'   = s + ALPHA*(-s + (1-s)*GAMMA*r)
When wr == jm*ones + (je-jm)*I exactly (checked host-side), s@wr.T is
computed exactly as jm*rowsum(s) + (je-jm)*s instead of a GEMM.

HW notes (probed on this runtime):
 - tensor_tensor_reduce faults the exec unit -> use tensor_mul+tensor_reduce.
 - DVE writes aliased through a bitcast (in-place f32r scale) fault -> avoided.
 - f32r is TF32; DVE/ACT may WRITE f32r (rounds) but reads go via .bitcast(F32).
"""

import os
import sys
import types

sys.path.insert(0, "/opt/trn_rl_repo")

import numpy as np

import concourse.bass as bass  # noqa: E402
import concourse.mybir as mybir  # noqa: E402
import concourse.tile as tile  # noqa: E402
from concourse import bacc  # noqa: E402

F32 = mybir.dt.float32
F32R = mybir.dt.float32r
AF = mybir.ActivationFunctionType
OP = mybir.AluOpType
AX = mybir.AxisListType

N_CORES = 8
B_FULL, I_DIM, H_DIM = 8192, 2048, 1024
BL = B_FULL // N_CORES

# Wong-Wang constants (match reference.py)
A_WW, B_WW, D_WW = 270.0, 108.0, 0.154
ALPHA, GAMMA = 1.0 / 100.0, 0.1
DA = D_WW * A_WW
DB = D_WW * B_WW
ZCLAMP = -87.0  # exp(z) below this is < 1.6e-38 ~ 0; keeps ACT LUT in range


def round_tf32(a: np.ndarray) -> np.ndarray:
    """fp32 -> tf32 (10-bit mantissa) round-to-nearest-even, fp32 container."""
    u = np.ascontiguousarray(a).view(np.uint32).astype(np.uint64)
    u = (u + 0x1000 + ((u >> 13) & 1)) & 0xFFFFE000
    return u.astype(np.uint32).view(np.float32)


def _chunks(n, size):
    out, i = [], 0
    while i < n:
        out.append((i, min(size, n - i)))
        i += size
    return out


def build_nc(
    n_cores=N_CORES,
    bl=BL,
    i_dim=I_DIM,
    h_dim=H_DIM,
    wr_struct=True,
    jm=-2.0,
    je=8.0,
):
    """Build + compile the SPMD bass program. bl is per-core batch rows.
    Requires bl%128==0, i_dim%512==0, h_dim%512==0, i_dim%n_cores==0."""
    P = 128
    NB = 512  # moving free dim / fp32 PSUM bank width
    bt_n = bl // P
    it_n = i_dim // P
    ic_n = i_dim // NB
    ht_n = h_dim // P
    hc_n = h_dim // NB
    sl_i = i_dim // n_cores
    inv_b = 1.0 / float(bl * n_cores)
    jejm = je - jm

    nc = bacc.Bacc(
        "TRN2", target_bir_lowering=False, debug=False, num_devices=n_cores
    )

    # ---- I/O ----
    xT_r = nc.dram_tensor("xT_r", [i_dim, bl], F32R, kind="ExternalInput").ap()
    x_n = nc.dram_tensor("x_n", [bl, i_dim], F32, kind="ExternalInput").ap()
    s_n = nc.dram_tensor("s_n", [bl, h_dim], F32, kind="ExternalInput").ap()
    y_n = nc.dram_tensor("y_n", [bl, h_dim], F32, kind="ExternalInput").ap()
    PT_r = nc.dram_tensor("PT_r", [i_dim, i_dim], F32R, kind="ExternalInput").ap()
    winT_r = nc.dram_tensor("winT_r", [i_dim, h_dim], F32R, kind="ExternalInput").ap()
    if not wr_struct:
        sT_r = nc.dram_tensor("sT_r", [h_dim, bl], F32R, kind="ExternalInput").ap()
        wrT_r = nc.dram_tensor(
            "wrT_r", [h_dim, h_dim], F32R, kind="ExternalInput"
        ).ap()
    PT_rows = nc.dram_tensor("PT_rows", [sl_i, i_dim], F32, kind="ExternalInput").ap()
    winT_rows = nc.dram_tensor(
        "winT_rows", [sl_i, h_dim], F32, kind="ExternalInput"
    ).ap()

    err_o = nc.dram_tensor("err_o", [bl, h_dim], F32, kind="ExternalOutput").ap()
    r_o = nc.dram_tensor("r_o", [bl, h_dim], F32, kind="ExternalOutput").ap()
    snew_o = nc.dram_tensor("snew_o", [bl, h_dim], F32, kind="ExternalOutput").ap()
    pnewT_o = nc.dram_tensor("pnewT_o", [sl_i, i_dim], F32, kind="ExternalOutput").ap()
    wnewT_o = nc.dram_tensor("wnewT_o", [sl_i, h_dim], F32, kind="ExternalOutput").ap()

    groups = [list(range(n_cores))]

    with tile.TileContext(nc) as tc:
        with (
            tc.tile_pool(name="res", bufs=1) as res,  # spans all phases
            tc.tile_pool(name="psum", bufs=8, space="PSUM") as psum,
            tc.tile_pool(name="dram", bufs=1, space="DRAM") as dram,
        ):
            kf_t = [
                res.tile([P, i_dim], F32R, tag=f"kf{bt}", name=f"kf{bt}")
                for bt in range(bt_n)
            ]
            errR_t = [
                res.tile([P, h_dim], F32R, tag=f"er{bt}", name=f"er{bt}")
                for bt in range(bt_n)
            ]
            rPr_t = [
                res.tile([P, 1], F32, tag=f"rPr{bt}", name=f"rPr{bt}")
                for bt in range(bt_n)
            ]
            d_t = [
                res.tile([P, 1], F32, tag=f"d{bt}", name=f"d{bt}")
                for bt in range(bt_n)
            ]
            if wr_struct:
                jms_t = [
                    res.tile([P, 1], F32, tag=f"jms{bt}", name=f"jms{bt}")
                    for bt in range(bt_n)
                ]

            # DRAM bounce buffers for collectives
            pp_part = dram.tile([i_dim, i_dim], F32, name="pp_part")
            dw_part = dram.tile([i_dim, h_dim], F32, name="dw_part")
            pp_red = dram.tile([sl_i, i_dim], F32, name="pp_red")
            dw_red = dram.tile([sl_i, h_dim], F32, name="dw_red")

            with tc.tile_pool(name="xTp", bufs=1) as xTp:
                xT_t = []
                for jt in range(it_n):
                    t = xTp.tile([P, bl], F32R, tag=f"xT{jt}", name=f"xT{jt}")
                    nc.sync.dma_start(t[:], xT_r[jt * P : (jt + 1) * P, :])
                    xT_t.append(t)

                # ------- Phase 1: kf = x@P.T; rPr; d; (jm*rowsum(s)) ---------
                with (
                    tc.tile_pool(name="ptp", bufs=2) as ptp,
                    tc.tile_pool(name="xnat", bufs=3) as xnat,
                    tc.tile_pool(name="scr1", bufs=2) as scr1,
                ):
                    if wr_struct:
                        # jm * rowsum(s) per batch tile via reduce over chunks
                        for bt in range(bt_n):
                            part = scr1.tile(
                                [P, hc_n], F32, tag="part", bufs=bt_n, name="part"
                            )
                            for hcc in range(hc_n):
                                s_t = xnat.tile([P, NB], F32, tag="ss", name="s_t")
                                nc.sync.dma_start(
                                    s_t[:],
                                    s_n[
                                        bt * P : (bt + 1) * P,
                                        hcc * NB : (hcc + 1) * NB,
                                    ],
                                )
                                nc.vector.tensor_reduce(
                                    part[:, hcc : hcc + 1], s_t[:], AX.X, OP.add
                                )
                            srow = scr1.tile(
                                [P, 1], F32, tag="srow", bufs=4, name="srow"
                            )
                            if hc_n == 1:
                                nc.vector.tensor_scalar_mul(
                                    jms_t[bt][:], part[:, 0:1], jm
                                )
                            else:
                                nc.vector.tensor_add(
                                    srow[:], part[:, 0:1], part[:, 1:2]
                                )
                                for hcc in range(2, hc_n):
                                    nc.vector.tensor_add(
                                        srow[:], srow[:], part[:, hcc : hcc + 1]
                                    )
                                nc.vector.tensor_scalar_mul(
                                    jms_t[bt][:], srow[:], jm
                                )

                    for ic in range(ic_n):
                        ps = [
                            psum.tile([P, NB], F32, tag="ps", name=f"ps_kf{ic}_{bt}")
                            for bt in range(bt_n)
                        ]
                        for j0, jn in _chunks(it_n, 4):
                            pt_t = ptp.tile([P, jn, NB], F32R, tag="pt", name="pt_t")
                            src = PT_r[
                                j0 * P : (j0 + jn) * P, ic * NB : (ic + 1) * NB
                            ].rearrange("(f p) n -> p f n", p=P)
                            nc.sync.dma_start(pt_t[:], src)
                            for j in range(jn):
                                jt = j0 + j
                                for bt in range(bt_n):
                                    nc.tensor.matmul(
                                        ps[bt][:],
                                        xT_t[jt][:, bt * P : (bt + 1) * P],
                                        pt_t[:, j, :],
                                        start=(jt == 0),
                                        stop=(jt == it_n - 1),
                                    )
                        for bt in range(bt_n):
                            kf_slice = kf_t[bt][:, ic * NB : (ic + 1) * NB]
                            nc.scalar.copy(kf_slice, ps[bt][:])
                            x_t = xnat.tile([P, NB], F32, tag="xn", name="x_t")
                            nc.sync.dma_start(
                                x_t[:],
                                x_n[bt * P : (bt + 1) * P, ic * NB : (ic + 1) * NB],
                            )
                            prod = scr1.tile([P, NB], F32, tag="dump", name="prod")
                            nc.vector.tensor_mul(
                                prod[:], kf_slice.bitcast(F32), x_t[:]
                            )
                            acc = scr1.tile(
                                [P, 1], F32, tag="acc", bufs=8, name="acc"
                            )
                            nc.vector.tensor_reduce(acc[:], prod[:], AX.X, OP.add)
                            if ic == 0:
                                nc.vector.tensor_scalar_add(
                                    rPr_t[bt][:], acc[:], 0.0
                                )
                            else:
                                nc.vector.tensor_add(
                                    rPr_t[bt][:], rPr_t[bt][:], acc[:]
                                )
                            if ic == ic_n - 1:
                                q1 = scr1.tile(
                                    [P, 1], F32, tag="q1", bufs=4, name="q1"
                                )
                                nc.vector.tensor_scalar_add(
                                    q1[:], rPr_t[bt][:], 1.0
                                )
                                nc.vector.reciprocal(d_t[bt][:], q1[:])

                # ------- Phase 2: rx = x@win.T (+ s@wr.T); epilogue ----------
                with (
                    tc.tile_pool(name="wq", bufs=2) as wq,
                    tc.tile_pool(name="ys", bufs=3) as ys,
                    tc.tile_pool(name="scr2", bufs=8) as scr2,
                    tc.tile_pool(name="outp2", bufs=5) as outp2,
                ):
                    for hc in range(hc_n):
                        ps = [
                            psum.tile([P, NB], F32, tag="ps", name=f"ps_rx{hc}_{bt}")
                            for bt in range(bt_n)
                        ]
                        for j0, jn in _chunks(it_n, 2):
                            w_t = wq.tile([P, jn, NB], F32R, tag="wq", name="w_t")
                            src = winT_r[
                                j0 * P : (j0 + jn) * P, hc * NB : (hc + 1) * NB
                            ].rearrange("(f p) n -> p f n", p=P)
                            nc.sync.dma_start(w_t[:], src)
                            for j in range(jn):
                                jt = j0 + j
                                stop = wr_struct and (jt == it_n - 1)
                                for bt in range(bt_n):
                                    nc.tensor.matmul(
                                        ps[bt][:],
                                        xT_t[jt][:, bt * P : (bt + 1) * P],
                                        w_t[:, j, :],
                                        start=(jt == 0),
                                        stop=stop,
                                    )
                        if not wr_struct:
                            for k0, kn in _chunks(ht_n, 2):
                                wr_t = wq.tile(
                                    [P, kn, NB], F32R, tag="wq", name="wr_t"
                                )
                                src = wrT_r[
                                    k0 * P : (k0 + kn) * P, hc * NB : (hc + 1) * NB
                                ].rearrange("(f p) n -> p f n", p=P)
                                nc.sync.dma_start(wr_t[:], src)
                                for k in range(kn):
                                    kt = k0 + k
                                    st_t = wq.tile(
                                        [P, bl], F32R, tag="st", bufs=3, name="st_t"
                                    )
                                    nc.sync.dma_start(
                                        st_t[:], sT_r[kt * P : (kt + 1) * P, :]
                                    )
                                    for bt in range(bt_n):
                                        nc.tensor.matmul(
                                            ps[bt][:],
                                            st_t[:, bt * P : (bt + 1) * P],
                                            wr_t[:, k, :],
                                            start=False,
                                            stop=(kt == ht_n - 1),
                                        )
                        # epilogue per batch tile
                        for bt in range(bt_n):
                            rows = slice(bt * P, (bt + 1) * P)
                            cols = slice(hc * NB, (hc + 1) * NB)
                            y_t = ys.tile([P, NB], F32, tag="y", name="y_t")
                            nc.sync.dma_start(y_t[:], y_n[rows, cols])
                            s_t = ys.tile([P, NB], F32, tag="s", name="s_t")
                            nc.sync.dma_start(s_t[:], s_n[rows, cols])

                            rx_t = scr2.tile([P, NB], F32, tag="sc", name="rx_t")
                            if wr_struct:
                                t1 = scr2.tile([P, NB], F32, tag="sc", name="t1")
                                nc.vector.tensor_scalar_mul(t1[:], s_t[:], jejm)
                                t2 = scr2.tile([P, NB], F32, tag="sc", name="t2")
                                nc.vector.tensor_scalar_add(
                                    t2[:], t1[:], jms_t[bt][:]
                                )
                                nc.vector.tensor_add(rx_t[:], ps[bt][:], t2[:])
                            else:
                                nc.scalar.copy(rx_t[:], ps[bt][:])

                            err_t = outp2.tile([P, NB], F32, tag="o", name="err_t")
                            nc.vector.tensor_sub(err_t[:], rx_t[:], y_t[:])
                            nc.sync.dma_start(err_o[rows, cols], err_t[:])
                            nc.vector.tensor_scalar_mul(
                                errR_t[bt][:, cols], err_t[:], 1.0
                            )

                            # r = z*ez/(D*(ez-1)), z = clamp(DA*rx - DB, >= -87)
                            z0 = scr2.tile([P, NB], F32, tag="sc", name="z0")
                            nc.vector.tensor_scalar(
                                z0[:], rx_t[:], DA, -DB, OP.mult, OP.add
                            )
                            z_t = scr2.tile([P, NB], F32, tag="sc", name="z_t")
                            nc.vector.tensor_scalar_max(z_t[:], z0[:], ZCLAMP)
                            ez_t = scr2.tile([P, NB], F32, tag="sc", name="ez_t")
                            nc.scalar.activation(
                                ez_t[:], z_t[:], AF.Exp, bias=0.0, scale=1.0
                            )
                            den_t = scr2.tile([P, NB], F32, tag="sc", name="den_t")
                            nc.vector.tensor_scalar(
                                den_t[:], ez_t[:], D_WW, -D_WW, OP.mult, OP.add
                            )
                            rden_t = scr2.tile([P, NB], F32, tag="sc", name="rden_t")
                            nc.vector.reciprocal(rden_t[:], den_t[:])
                            num_t = scr2.tile([P, NB], F32, tag="sc", name="num_t")
                            nc.vector.tensor_mul(num_t[:], z_t[:], ez_t[:])
                            r_t = outp2.tile([P, NB], F32, tag="o", name="r_t")
                            nc.vector.tensor_mul(r_t[:], num_t[:], rden_t[:])
                            nc.sync.dma_start(r_o[rows, cols], r_t[:])

                            # s' = s + ALPHA*(GAMMA*(1-s)*r - s)
                            a_t = scr2.tile([P, NB], F32, tag="sc", name="a_t")
                            nc.scalar.activation(
                                a_t[:], s_t[:], AF.Identity, bias=1.0, scale=-1.0
                            )
                            b_t = scr2.tile([P, NB], F32, tag="sc", name="b_t")
                            nc.vector.tensor_mul(b_t[:], a_t[:], r_t[:])
                            g2_t = scr2.tile([P, NB], F32, tag="sc", name="g2_t")
                            nc.scalar.mul(g2_t[:], b_t[:], GAMMA)
                            t3 = scr2.tile([P, NB], F32, tag="sc", name="t3")
                            nc.vector.tensor_sub(t3[:], g2_t[:], s_t[:])
                            e_t = scr2.tile([P, NB], F32, tag="sc", name="e_t")
                            nc.scalar.mul(e_t[:], t3[:], ALPHA)
                            sn_t = outp2.tile([P, NB], F32, tag="o", name="sn_t")
                            nc.vector.tensor_add(sn_t[:], e_t[:], s_t[:])
                            nc.sync.dma_start(snew_o[rows, cols], sn_t[:])

            # ---------- Phase 3: k = kf*d; pp = k.T@kf; dw = k.T@errR -------
            with (
                tc.tile_pool(name="kp", bufs=1) as kp,
                tc.tile_pool(name="outp3", bufs=6) as outp3,
            ):
                k_t = [
                    kp.tile([P, i_dim], F32R, tag=f"k{bt}", name=f"k{bt}")
                    for bt in range(bt_n)
                ]
                for ic in range(ic_n):
                    sl = slice(ic * NB, (ic + 1) * NB)
                    for bt in range(bt_n):
                        nc.vector.tensor_scalar_mul(
                            k_t[bt][:, sl], kf_t[bt][:, sl].bitcast(F32), d_t[bt][:]
                        )
                for mt in range(it_n):
                    mcols = slice(mt * P, (mt + 1) * P)
                    for nc4 in range(ic_n):
                        ncols = slice(nc4 * NB, (nc4 + 1) * NB)
                        ps_b = psum.tile(
                            [P, NB], F32, tag="ps", name=f"ps_pp{mt}_{nc4}"
                        )
                        for bt in range(bt_n):
                            nc.tensor.matmul(
                                ps_b[:],
                                k_t[bt][:, mcols],
                                kf_t[bt][:, ncols],
                                start=(bt == 0),
                                stop=(bt == bt_n - 1),
                            )
                        o_t = outp3.tile([P, NB], F32, tag="o", name="pp_o")
                        nc.any.tensor_copy(o_t[:], ps_b[:])
                        nc.sync.dma_start(pp_part[mcols, ncols], o_t[:])
                for mt in range(it_n):
                    mcols = slice(mt * P, (mt + 1) * P)
                    for nc2 in range(hc_n):
                        ncols = slice(nc2 * NB, (nc2 + 1) * NB)
                        ps_b = psum.tile(
                            [P, NB], F32, tag="ps", name=f"ps_dw{mt}_{nc2}"
                        )
                        for bt in range(bt_n):
                            nc.tensor.matmul(
                                ps_b[:],
                                k_t[bt][:, mcols],
                                errR_t[bt][:, ncols],
                                start=(bt == 0),
                                stop=(bt == bt_n - 1),
                            )
                        o_t = outp3.tile([P, NB], F32, tag="o", name="dw_o")
                        nc.any.tensor_copy(o_t[:], ps_b[:])
                        nc.sync.dma_start(dw_part[mcols, ncols], o_t[:])

            # ---------- Phase 4: reduce-scatter + finalize ------------------
            nc.gpsimd.collective_compute(
                "ReduceScatter",
                OP.add,
                replica_groups=groups,
                ins=[pp_part[:].opt()],
                outs=[pp_red[:].opt()],
            )
            nc.gpsimd.collective_compute(
                "ReduceScatter",
                OP.add,
                replica_groups=groups,
                ins=[dw_part[:].opt()],
                outs=[dw_red[:].opt()],
            )

            with tc.tile_pool(name="fin", bufs=2) as fin:
                for t0, tn in _chunks(sl_i, P):
                    rows = slice(t0, t0 + tn)
                    pt_x = fin.tile([P, i_dim], F32, tag="fa", name="pt_x")
                    rd_x = fin.tile([P, i_dim], F32, tag="fb", name="rd_x")
                    rd_s = fin.tile([P, i_dim], F32, tag="fbs", name="rd_s")
                    po_x = fin.tile([P, i_dim], F32, tag="fao", name="po_x")
                    nc.sync.dma_start(pt_x[:tn], PT_rows[rows, :])
                    nc.sync.dma_start(rd_x[:tn], pp_red[rows, :])
                    nc.vector.tensor_scalar_mul(rd_s[:tn], rd_x[:tn], inv_b)
                    nc.vector.tensor_sub(po_x[:tn], pt_x[:tn], rd_s[:tn])
                    nc.sync.dma_start(pnewT_o[rows, :], po_x[:tn])

                    wt_x = fin.tile([P, h_dim], F32, tag="fc", name="wt_x")
                    wd_x = fin.tile([P, h_dim], F32, tag="fd", name="wd_x")
                    wd_s = fin.tile([P, h_dim], F32, tag="fds", name="wd_s")
                    wo_x = fin.tile([P, h_dim], F32, tag="fco", name="wo_x")
                    nc.sync.dma_start(wt_x[:tn], winT_rows[rows, :])
                    nc.sync.dma_start(wd_x[:tn], dw_red[rows, :])
                    nc.vector.tensor_scalar_mul(wd_s[:tn], wd_x[:tn], inv_b)
                    nc.vector.tensor_sub(wo_x[:tn], wt_x[:tn], wd_s[:tn])
                    nc.sync.dma_start(wnewT_o[rows, :], wo_x[:tn])

    nc.compile()
    return nc


def wr_structure(wr: np.ndarray):
    """If wr == jm*ones + (je-jm)*I exactly, return (jm, je); else None."""
    if wr.shape[0] != wr.shape[1]:
        return None
    jm = float(wr[0, 1])
    je = float(wr[0, 0])
    expect = np.where(np.eye(wr.shape[0], dtype=bool), np.float32(je), np.float32(jm))
    if np.array_equal(wr, expect):
        return jm, je
    return None


def make_in_maps(x, s, y, P, win, wr, n_cores=N_CORES, wr_struct=True):
    bl = x.shape[0] // n_cores
    i_dim = x.shape[1]
    sl_i = i_dim // n_cores
    PT = np.ascontiguousarray(P.T)
    winT = np.ascontiguousarray(win.T)
    PT_r = round_tf32(PT)
    winT_r = round_tf32(winT)
    if not wr_struct:
        wrT_r = round_tf32(np.ascontiguousarray(wr.T))
    in_maps = []
    for c in range(n_cores):
        rows = slice(c * bl, (c + 1) * bl)
        srl = slice(c * sl_i, (c + 1) * sl_i)
        m = {
            "xT_r": round_tf32(x[rows].T),
            "x_n": np.ascontiguousarray(x[rows]),
            "s_n": np.ascontiguousarray(s[rows]),
            "y_n": np.ascontiguousarray(y[rows]),
            "PT_r": PT_r,
            "winT_r": winT_r,
            "PT_rows": np.ascontiguousarray(PT[srl]),
            "winT_rows": np.ascontiguousarray(winT[srl]),
        }
        if not wr_struct:
            m["sT_r"] = round_tf32(s[rows].T)
            m["wrT_r"] = wrT_r
        in_maps.append(m)
    return in_maps


def assemble_outputs(results):
    err = np.concatenate([r["err_o"] for r in results], axis=0)
    r_out = np.concatenate([r["r_o"] for r in results], axis=0)
    s_new = np.concatenate([r["snew_o"] for r in results], axis=0)
    P_newT = np.concatenate([r["pnewT_o"] for r in results], axis=0)
    win_newT = np.concatenate([r["wnewT_o"] for r in results], axis=0)
    P_new = np.ascontiguousarray(P_newT.T)
    win_new = np.ascontiguousarray(win_newT.T)
    return err, r_out, s_new, P_new, win_new


def _install_ntff_hook_shim():
    """Register the axon NTFF profile hook if the image's antenv lacks it
    (needed only when tracing; harmless otherwise)."""
    try:
        from antenv import axon_hooks  # noqa: F401
        return True
    except ImportError:
        pass
    try:
        import antenv
        mod = types.ModuleType("antenv.axon_hooks")
        _hook = [None]
        mod.set_axon_ntff_profile_hook = lambda h: _hook.__setitem__(0, h)
        mod.get_axon_ntff_profile_hook = lambda: _hook[0]
        sys.modules["antenv.axon_hooks"] = mod
        antenv.axon_hooks = mod
        from trn_agent_boot.trn_boot import _ntff_profile_via_ctypes
        h = _ntff_profile_via_ctypes("/opt/axon/libaxon_pjrt.so")
        if h is None:
            return False
        mod.set_axon_ntff_profile_hook(h)
        return True
    except Exception:
        return False


_NC_CACHE = {}
LAST_RESULT = None


def kernel(x, s, y, P, win, wr):
    global LAST_RESULT
    from concourse.bass_utils import run_bass_kernel_spmd

    x = np.asarray(x, dtype=np.float32)
    s = np.asarray(s, dtype=np.float32)
    y = np.asarray(y, dtype=np.float32)
    P = np.asarray(P, dtype=np.float32)
    win = np.asarray(win, dtype=np.float32)
    wr = np.asarray(wr, dtype=np.float32)

    st = wr_structure(wr)
    jm, je = st if st is not None else (0.0, 0.0)
    key = (x.shape, s.shape, st is not None, jm, je)
    if key not in _NC_CACHE:
        _NC_CACHE[key] = build_nc(
            n_cores=N_CORES,
            bl=x.shape[0] // N_CORES,
            i_dim=x.shape[1],
            h_dim=s.shape[1],
            wr_struct=st is not None,
            jm=jm,
            je=je,
        )
    nc = _NC_CACHE[key]

    in_maps = make_in_maps(x, s, y, P, win, wr, wr_struct=st is not None)
    trace = os.environ.get("KERNEL_TRACE", "0") == "1"
    if trace:
        trace = _install_ntff_hook_shim()
    res = run_bass_kernel_spmd(
        nc, in_maps, core_ids=list(range(N_CORES)), trace=trace
    )
    LAST_RESULT = res
    return assemble_outputs(res.results)
